# revision 1
# baseline (speedup 1.0000x reference)
"""Multi-head causal attention with RoPE on 8 Trainium2 NeuronCores.

Sharding: tensor-parallel over heads. Each core owns 2 of the 16 heads
(a 128-wide slice of D_OUT): it computes Q/K/V projections for its slice
(column-sliced Wq/Wk/Wv), RoPE, causal attention, and a row-sliced
out-projection partial. The 8 partials are summed on the host (the
all-reduce of the row-parallel out projection) and the bias added once.

Emission is software-pipelined for the in-order PE: the next window's
projection matmuls are interleaved between exp and ctx inside the
attention loop (covering ACT latency), and each window's out-projection
is delayed one window so PE never waits on the normalize chain.
"""

import sys

sys.path.insert(0, "/opt/trn_rl_repo")

from contextlib import ExitStack

import numpy as np

import concourse.bass as bass
import concourse.tile as tile
from concourse import bacc, mybir
from concourse.bass import MemorySpace
from concourse.bass_utils import run_bass_kernel_spmd

B, T, D, H, DH = 2, 2048, 1024, 16, 64
NCORES = 8
DC = D // NCORES  # 128: d-slice per core (2 heads)
QSB = 512  # query superblock
NW = T // QSB  # windows per batch

f32 = mybir.dt.float32
f32r = mybir.dt.float32r
AF = mybir.ActivationFunctionType

SWAP_MASK = []
for _i in range(16):
    SWAP_MASK += [2 * _i + 1, 2 * _i]

_CACHE = {}


def _build(reps=1, pipeline=True, delay_out=False, merge_out=False, act_frac=2, qt_bufs=2, rope_bufs=6, s_bufs=3, ctx_bufs=2, ost_bufs=4, p_bufs=6):
    nc = bacc.Bacc("TRN2", target_bir_lowering=False, debug=False)
    xT = nc.dram_tensor("xt", [B, D, T], f32r, kind="ExternalInput").ap()
    wq = nc.dram_tensor("wq", [D, DC], f32r, kind="ExternalInput").ap()
    wk = nc.dram_tensor("wk", [D, DC], f32r, kind="ExternalInput").ap()
    wv = nc.dram_tensor("wv", [D, DC], f32r, kind="ExternalInput").ap()
    wo = nc.dram_tensor("wo", [DC, D], f32r, kind="ExternalInput").ap()
    ropec = nc.dram_tensor("ropec", [DC, T], f32, kind="ExternalInput").ap()
    ropes = nc.dram_tensor("ropes", [DC, T], f32, kind="ExternalInput").ap()
    mneg = nc.dram_tensor("mneg", [128, 128], f32, kind="ExternalInput").ap()
    ident = nc.dram_tensor("ident", [128, 128], f32, kind="ExternalInput").ap()
    vones = nc.dram_tensor("vones", [128, 16], f32r, kind="ExternalInput").ap()
    out = nc.dram_tensor("out", [B, T, D], f32, kind="ExternalOutput").ap()

    with tile.TileContext(nc) as tc, ExitStack() as ctx:
        const = ctx.enter_context(tc.tile_pool(name="const", bufs=1))
        x_pool = ctx.enter_context(tc.tile_pool(name="x_pool", bufs=2))
        kt_pool = ctx.enter_context(tc.tile_pool(name="kt_pool", bufs=2))
        va_pool = ctx.enter_context(tc.tile_pool(name="va_pool", bufs=2))
        qt_pool = ctx.enter_context(tc.tile_pool(name="qt_pool", bufs=qt_bufs))
        rope_pool = ctx.enter_context(tc.tile_pool(name="rope_pool", bufs=rope_bufs))
        vt_pool = ctx.enter_context(tc.tile_pool(name="vt_pool", bufs=2))
        p_pool = ctx.enter_context(tc.tile_pool(name="p_pool", bufs=p_bufs))
        nrm_pool = ctx.enter_context(tc.tile_pool(name="nrm_pool", bufs=2))
        ctxn_pool = ctx.enter_context(tc.tile_pool(name="ctxn_pool", bufs=2))
        ost_pool = ctx.enter_context(tc.tile_pool(name="ost_pool", bufs=ost_bufs))

        proj_ps = ctx.enter_context(
            tc.tile_pool(
                name="proj_ps", bufs=(1 if merge_out else 2), space=MemorySpace.PSUM
            )
        )
        s_ps = ctx.enter_context(
            tc.tile_pool(name="s_ps", bufs=s_bufs, space=MemorySpace.PSUM)
        )
        ctx_ps = ctx.enter_context(
            tc.tile_pool(name="ctx_ps", bufs=ctx_bufs, space=MemorySpace.PSUM)
        )
        out_ps = ctx.enter_context(
            tc.tile_pool(name="out_ps", bufs=1, space=MemorySpace.PSUM)
        )

        # ---- constants (first-use order; first window's loads split) ----
        wq_sb = const.tile([128, 8, DC], f32r)
        wk_sb = const.tile([128, 8, DC], f32r)
        wv_sb = const.tile([128, 8, DC], f32r)
        wqr = wq.rearrange("(c p) m -> p c m", p=128)
        xw0 = x_pool.tile([128, 8, QSB], f32r, name="xw0", tag="xw")
        xr0 = xT[0, :, 0:QSB].rearrange("(c p) t -> p c t", p=128)
        for kc in range(8):
            nc.sync.dma_start(out=wq_sb[:, kc : kc + 1], in_=wqr[:, kc : kc + 1])
            nc.sync.dma_start(out=xw0[:, kc : kc + 1], in_=xr0[:, kc : kc + 1])
        nc.sync.dma_start(out=wk_sb, in_=wk.rearrange("(c p) m -> p c m", p=128))
        ct_sb = const.tile([128, T], f32)
        st_sb = const.tile([128, T], f32)
        nc.sync.dma_start(out=ct_sb, in_=ropec)
        nc.sync.dma_start(out=st_sb, in_=ropes)
        nc.sync.dma_start(out=wv_sb, in_=wv.rearrange("(c p) m -> p c m", p=128))
        ident_sb = const.tile([128, 128], f32)
        nc.sync.dma_start(out=ident_sb, in_=ident)
        mneg_sb = const.tile([128, 128], f32)
        nc.sync.dma_start(out=mneg_sb, in_=mneg)
        wo_sb = const.tile([128, D], f32r)
        nc.sync.dma_start(out=wo_sb, in_=wo)

        for rep in range(reps):
            KTs, VAs = {}, {}

            def prep_window(b, w, first):
                """Emit xw DMA now; return (qt_tile, quanta closures)."""
                t0 = w * QSB
                if w == 0:
                    KTs[b] = kt_pool.tile([128, T], f32r, name=f"KT{b}", tag="KT")
                    VAs[b] = va_pool.tile(
                        [128, 16, 130], f32r, name=f"VA{b}", tag="VA"
                    )
                    nc.sync.dma_start(
                        out=VAs[b][:, :, 64:65],
                        in_=vones.rearrange("p (c o) -> p c o", o=1),
                    )
                    nc.sync.dma_start(
                        out=VAs[b][:, :, 129:130],
                        in_=vones.rearrange("p (c o) -> p c o", o=1),
                    )
                KT, VA = KTs[b], VAs[b]
                if first:
                    xw = xw0
                else:
                    xw = x_pool.tile([128, 8, QSB], f32r, name="xw", tag="xw")
                    nc.sync.dma_start(
                        out=xw,
                        in_=xT[b, :, t0 : t0 + QSB].rearrange(
                            "(c p) t -> p c t", p=128
                        ),
                    )
                qt = qt_pool.tile([128, QSB], f32r, name="qt")
                state = {}

                def mk_proj(wsb, key):
                    def run():
                        ps = proj_ps.tile([128, QSB], f32, tag="proj", name="ps")
                        for kc in range(8):
                            nc.tensor.matmul(
                                ps,
                                wsb[:, kc],
                                xw[:, kc],
                                start=(kc == 0),
                                stop=(kc == 7),
                            )
                        state[key] = ps

                    return run

                def mk_rope(key, dest_fn):
                    def run():
                        ps = state.pop(key)
                        dest = dest_fn()
                        sh = rope_pool.tile([128, QSB], f32, tag="ropet", name="sh")
                        nc.vector.stream_shuffle(sh, ps, SWAP_MASK)
                        m1 = rope_pool.tile([128, QSB], f32, tag="ropet", name="m1")
                        nc.vector.tensor_mul(m1, ps, ct_sb[:, t0 : t0 + QSB])
                        m2 = rope_pool.tile([128, QSB], f32, tag="ropet", name="m2")
                        nc.vector.tensor_mul(m2, sh, st_sb[:, t0 : t0 + QSB])
                        nc.gpsimd.tensor_add(dest, m1, m2)

                    return run

                def mk_vpath():
                    def run():
                        ps = state.pop("v")
                        vt = vt_pool.tile([128, QSB], f32, name="vt")
                        nc.vector.tensor_copy(vt, ps)
                        vblk = proj_ps.tile([128, 4, 128], f32, tag="proj", name="vb")
                        for i in range(4):
                            nc.tensor.transpose(
                                vblk[:, i], vt[:, 128 * i : 128 * i + 128], ident_sb
                            )
                        for i in range(4):
                            tb = 4 * w + i
                            nc.vector.tensor_copy(VA[:, tb, 0:64], vblk[:, i, 0:64])
                            nc.vector.tensor_copy(
                                VA[:, tb, 65:129], vblk[:, i, 64:128]
                            )

                    return run

                quanta = [
                    mk_proj(wq_sb, "q"),
                    mk_rope("q", lambda: qt),
                    mk_proj(wk_sb, "k"),
                    mk_rope("k", lambda: KT[:, t0 : t0 + QSB]),
                    mk_proj(wv_sb, "v"),
                    mk_vpath(),
                ]
                return qt, quanta

            def emit_outproj(ctxn, b, t0):
                for ts in range(4):
                    if merge_out:
                        ops = out_ps.tile([128, 2, 512], f32, name="ops", tag="ops")
                        for eh in range(2):
                            nc.tensor.matmul(
                                ops[:, eh],
                                ctxn[:, 128 * ts : 128 * ts + 128],
                                wo_sb[:, 512 * eh : 512 * eh + 512],
                                start=True,
                                stop=True,
                            )
                        ost = ost_pool.tile([128, 2, 512], f32, name="ost", tag="ost")
                        if ts % 2 == 0:
                            nc.vector.tensor_copy(ost, ops)
                        else:
                            nc.scalar.copy(ost, ops)
                        nc.sync.dma_start(
                            out=out[b, t0 + 128 * ts : t0 + 128 * ts + 128, :],
                            in_=ost.rearrange("p a n -> p (a n)"),
                        )
                    else:
                        for eh in range(2):
                            ops = out_ps.tile([128, 512], f32, name="ops", tag="ops")
                            nc.tensor.matmul(
                                ops,
                                ctxn[:, 128 * ts : 128 * ts + 128],
                                wo_sb[:, 512 * eh : 512 * eh + 512],
                                start=True,
                                stop=True,
                            )
                            ost = ost_pool.tile([128, 512], f32, name="ost", tag="ost")
                            if eh == 0:
                                nc.vector.tensor_copy(ost, ops)
                            else:
                                nc.scalar.copy(ost, ops)
                            nc.sync.dma_start(
                                out=out[
                                    b,
                                    t0 + 128 * ts : t0 + 128 * ts + 128,
                                    512 * eh : 512 * eh + 512,
                                ],
                                in_=ost,
                            )

            seq = [(b, w) for b in range(B) for w in range(NW)]
            qts = {}
            qt0, quanta0 = prep_window(0, 0, first=(rep == 0))
            for q in quanta0:
                q()
            qts[(0, 0)] = qt0
            pending = None

            for i, (b, w) in enumerate(seq):
                t0 = w * QSB
                KT, VA = KTs[b], VAs[b]
                qt = qts.pop((b, w))

                nq = []
                if i + 1 < len(seq):
                    nb, nw = seq[i + 1]
                    qtn, nq = prep_window(nb, nw, first=False)
                    qts[(nb, nw)] = qtn
                if not pipeline:
                    for q in nq:
                        q()
                    nq = []

                nkb = 4 * w + 4
                cps = [
                    ctx_ps.tile([65, QSB], f32, tag="ctx", name=f"cps{h}")
                    for h in range(2)
                ]
                emitted = 0
                for kb in range(nkb):
                    o = kb - 4 * w
                    col0 = 128 * o if o > 0 else 0
                    ncols = QSB - col0
                    pts = []
                    for h in range(2):
                        sps = s_ps.tile([128, QSB], f32, tag="s", name="sps")
                        nc.tensor.matmul(
                            sps[:, :ncols],
                            KT[64 * h : 64 * h + 64, 128 * kb : 128 * kb + 128],
                            qt[64 * h : 64 * h + 64, col0:QSB],
                            start=True,
                            stop=True,
                        )
                        if o >= 0:
                            nc.vector.tensor_add(
                                sps[:, 0:128], sps[:, 0:128], mneg_sb
                            )
                        pt = p_pool.tile([128, QSB], f32r, tag="pt", name="pt")
                        nc.scalar.activation(
                            pt[:, :ncols], sps[:, :ncols], AF.Exp, scale=0.125
                        )
                        pts.append(pt)
                    # interleave next-window projection quanta while ACT runs exp
                    want = (kb + 1) * len(nq) // nkb
                    while emitted < want:
                        nq[emitted]()
                        emitted += 1
                    for h in range(2):
                        nc.tensor.matmul(
                            cps[h][:, col0:QSB],
                            VA[:, kb, 65 * h : 65 * h + 65],
                            pts[h][:, :ncols],
                            start=(kb == 0),
                            stop=(kb == nkb - 1),
                        )
                while emitted < len(nq):
                    nq[emitted]()
                    emitted += 1

                # out-projection of the PREVIOUS window (deps long satisfied)
                if delay_out and pending is not None:
                    emit_outproj(*pending)
                    pending = None

                # normalize this window
                ctxn = ctxn_pool.tile([128, QSB], f32r, name="ctxn")
                for h in range(2):
                    rc = nrm_pool.tile([65, QSB], f32, tag="rc", name="rc")
                    nc.vector.reciprocal(rc[64:65, :], cps[h][64:65, :])
                    rc0 = nrm_pool.tile([1, QSB], f32, tag="rc0", name="rc0")
                    nc.sync.dma_start(out=rc0, in_=rc[64:65, :])
                    bc = nrm_pool.tile([64, QSB], f32, tag="bc", name="bc")
                    nc.gpsimd.partition_broadcast(bc, rc0, channels=64)
                    if h == 0:
                        nc.vector.tensor_mul(ctxn[0:64, :], cps[h][0:64, :], bc)
                    else:
                        cn1 = nrm_pool.tile([64, QSB], f32r, tag="cn1", name="cn1")
                        nc.vector.tensor_mul(cn1, cps[h][0:64, :], bc)
                        nc.sync.dma_start(out=ctxn[64:128, :], in_=cn1)
                if delay_out:
                    pending = (ctxn, b, t0)
                else:
                    emit_outproj(ctxn, b, t0)

            if pending is not None:
                emit_outproj(*pending)

    nc.compile()
    return nc


def _host_inputs(x, Wq, Wk, Wv, Wo):
    xT = np.ascontiguousarray(x.transpose(0, 2, 1))

    pos = np.arange(T, dtype=np.float64)
    inv_freq = np.power(10000.0, -2.0 * np.arange(0, DH, 2) / DH)  # (32,)
    freqs = pos[:, None] * inv_freq[None, :]  # (T, 32)
    cos = np.cos(freqs)
    sin = np.sin(freqs)
    ct = np.empty((DC, T), np.float32)
    st = np.empty((DC, T), np.float32)
    for p in range(DC):
        i = (p % DH) // 2
        ct[p] = cos[:, i]
        st[p] = sin[:, i] * (-1.0 if p % 2 == 0 else 1.0)

    pp, cc = np.meshgrid(np.arange(128), np.arange(128), indexing="ij")
    mneg = np.where(pp <= cc, 0.0, -1e9).astype(np.float32)
    ident = np.eye(128, dtype=np.float32)

    per_core = []
    for c in range(NCORES):
        sl = slice(c * DC, (c + 1) * DC)
        per_core.append(
            {
                "xt": xT,
                "wq": np.ascontiguousarray(Wq[:, sl]),
                "wk": np.ascontiguousarray(Wk[:, sl]),
                "wv": np.ascontiguousarray(Wv[:, sl]),
                "wo": np.ascontiguousarray(Wo[sl, :]),
                "ropec": ct,
                "ropes": st,
                "mneg": mneg,
                "ident": ident,
                "vones": np.ones((128, 16), np.float32),
            }
        )
    return per_core


def kernel(x, Wq, Wk, Wv, Wo, bo):
    x = np.asarray(x, np.float32)
    Wq = np.asarray(Wq, np.float32)
    Wk = np.asarray(Wk, np.float32)
    Wv = np.asarray(Wv, np.float32)
    Wo = np.asarray(Wo, np.float32)
    bo = np.asarray(bo, np.float32)

    if "nc" not in _CACHE:
        _CACHE["nc"] = _build()
    nc = _CACHE["nc"]

    in_maps = _host_inputs(x, Wq, Wk, Wv, Wo)
    res = run_bass_kernel_spmd(nc, in_maps, list(range(NCORES)))
    acc = res.results[0]["out"].astype(np.float64)
    for c in range(1, NCORES):
        acc += res.results[c]["out"]
    acc += bo.astype(np.float64)
    return acc.astype(np.float32)



# revision 7
# speedup vs baseline: 2.3320x; 2.3320x over previous
"""Multi-head causal attention with RoPE on 8 Trainium2 NeuronCores.

Sharding: tensor-parallel over heads. Each core owns 2 of the 16 heads
(a 128-wide slice of D_OUT): it computes Q/K/V projections for its slice
(column-sliced Wq/Wk/Wv), RoPE, causal attention, and a row-sliced
out-projection partial. The 8 partials are summed on the host (the
all-reduce of the row-parallel out projection) and the bias added once.

Emission is software-pipelined for the in-order PE: the next window's
projection matmuls are interleaved between exp and ctx inside the
attention loop (covering ACT latency), and each window's out-projection
is delayed one window so PE never waits on the normalize chain.
"""

import sys

sys.path.insert(0, "/opt/trn_rl_repo")

from contextlib import ExitStack

import numpy as np

import concourse.bass as bass
import concourse.tile as tile
from concourse import bacc, mybir
from concourse.bass import MemorySpace
from concourse.bass_utils import run_bass_kernel_spmd

B, T, D, H, DH = 2, 2048, 1024, 16, 64
NCORES = 8
DC = D // NCORES  # 128: d-slice per core (2 heads)
QSB = 512  # query superblock
NW = T // QSB  # windows per batch

f32 = mybir.dt.float32
f32r = mybir.dt.float32r
f16 = mybir.dt.float16
AF = mybir.ActivationFunctionType

SWAP_MASK = []
for _i in range(16):
    SWAP_MASK += [2 * _i + 1, 2 * _i]

_CACHE = {}


def _build(reps=1, pipeline=True, delay_out=False, merge_out=False, act_frac=2, qt_bufs=2, rope_bufs=6, s_bufs=3, ctx_bufs=2, ost_bufs=4, p_bufs=6):
    nc = bacc.Bacc("TRN2", target_bir_lowering=False, debug=False)
    xT = nc.dram_tensor("xt", [B, D, T], f16, kind="ExternalInput").ap()
    wq = nc.dram_tensor("wq", [D, DC], f16, kind="ExternalInput").ap()
    wk = nc.dram_tensor("wk", [D, DC], f16, kind="ExternalInput").ap()
    wv = nc.dram_tensor("wv", [D, DC], f16, kind="ExternalInput").ap()
    wo = nc.dram_tensor("wo", [DC, D], f16, kind="ExternalInput").ap()
    ropec = nc.dram_tensor("ropec", [DC, T], f32, kind="ExternalInput").ap()
    ropes = nc.dram_tensor("ropes", [DC, T], f32, kind="ExternalInput").ap()
    mneg = nc.dram_tensor("mneg", [128, 128], f32, kind="ExternalInput").ap()
    ident = nc.dram_tensor("ident", [128, 128], f16, kind="ExternalInput").ap()
    vones = nc.dram_tensor("vones", [128, 16], f16, kind="ExternalInput").ap()
    out = nc.dram_tensor("out", [B, T, D], f16, kind="ExternalOutput").ap()

    with tile.TileContext(nc) as tc, ExitStack() as ctx:
        const = ctx.enter_context(tc.tile_pool(name="const", bufs=1))
        x_pool = ctx.enter_context(tc.tile_pool(name="x_pool", bufs=2))
        kt_pool = ctx.enter_context(tc.tile_pool(name="kt_pool", bufs=2))
        va_pool = ctx.enter_context(tc.tile_pool(name="va_pool", bufs=2))
        qt_pool = ctx.enter_context(tc.tile_pool(name="qt_pool", bufs=qt_bufs))
        rope_pool = ctx.enter_context(tc.tile_pool(name="rope_pool", bufs=rope_bufs))
        vt_pool = ctx.enter_context(tc.tile_pool(name="vt_pool", bufs=2))
        p_pool = ctx.enter_context(tc.tile_pool(name="p_pool", bufs=p_bufs))
        nrm_pool = ctx.enter_context(tc.tile_pool(name="nrm_pool", bufs=2))
        ctxn_pool = ctx.enter_context(tc.tile_pool(name="ctxn_pool", bufs=2))
        ost_pool = ctx.enter_context(tc.tile_pool(name="ost_pool", bufs=ost_bufs))

        proj_ps = ctx.enter_context(
            tc.tile_pool(
                name="proj_ps", bufs=(1 if merge_out else 2), space=MemorySpace.PSUM
            )
        )
        s_ps = ctx.enter_context(
            tc.tile_pool(name="s_ps", bufs=s_bufs, space=MemorySpace.PSUM)
        )
        ctx_ps = ctx.enter_context(
            tc.tile_pool(name="ctx_ps", bufs=ctx_bufs, space=MemorySpace.PSUM)
        )
        out_ps = ctx.enter_context(
            tc.tile_pool(name="out_ps", bufs=1, space=MemorySpace.PSUM)
        )

        # ---- constants (first-use order; first window's loads split) ----
        wq_sb = const.tile([128, 8, DC], f16)
        wk_sb = const.tile([128, 8, DC], f16)
        wv_sb = const.tile([128, 8, DC], f16)
        wqr = wq.rearrange("(c p) m -> p c m", p=128)
        xw0 = x_pool.tile([128, 8, QSB], f16, name="xw0", tag="xw")
        xr0 = xT[0, :, 0:QSB].rearrange("(c p) t -> p c t", p=128)
        for kc in range(8):
            nc.sync.dma_start(out=wq_sb[:, kc : kc + 1], in_=wqr[:, kc : kc + 1])
            nc.sync.dma_start(out=xw0[:, kc : kc + 1], in_=xr0[:, kc : kc + 1])
        nc.sync.dma_start(out=wk_sb, in_=wk.rearrange("(c p) m -> p c m", p=128))
        ct_sb = const.tile([128, T], f32)
        st_sb = const.tile([128, T], f32)
        nc.sync.dma_start(out=ct_sb, in_=ropec)
        nc.sync.dma_start(out=st_sb, in_=ropes)
        nc.sync.dma_start(out=wv_sb, in_=wv.rearrange("(c p) m -> p c m", p=128))
        ident_sb = const.tile([128, 128], f16)
        nc.sync.dma_start(out=ident_sb, in_=ident)
        mneg_sb = const.tile([128, 128], f32)
        nc.sync.dma_start(out=mneg_sb, in_=mneg)
        wo_sb = const.tile([128, D], f16)
        nc.sync.dma_start(out=wo_sb, in_=wo)

        for rep in range(reps):
            KTs, VAs = {}, {}

            def prep_window(b, w, first):
                """Emit xw DMA now; return (qt_tile, quanta closures)."""
                t0 = w * QSB
                if w == 0:
                    KTs[b] = kt_pool.tile([128, T], f16, name=f"KT{b}", tag="KT")
                    VAs[b] = va_pool.tile(
                        [128, 16, 130], f16, name=f"VA{b}", tag="VA"
                    )
                    nc.sync.dma_start(
                        out=VAs[b][:, :, 64:65],
                        in_=vones.rearrange("p (c o) -> p c o", o=1),
                    )
                    nc.sync.dma_start(
                        out=VAs[b][:, :, 129:130],
                        in_=vones.rearrange("p (c o) -> p c o", o=1),
                    )
                KT, VA = KTs[b], VAs[b]
                if first:
                    xw = xw0
                else:
                    xw = x_pool.tile([128, 8, QSB], f16, name="xw", tag="xw")
                    nc.sync.dma_start(
                        out=xw,
                        in_=xT[b, :, t0 : t0 + QSB].rearrange(
                            "(c p) t -> p c t", p=128
                        ),
                    )
                qt = qt_pool.tile([128, QSB], f16, name="qt")
                state = {}

                def mk_proj(wsb, key):
                    def run():
                        ps = proj_ps.tile([128, QSB], f32, tag="proj", name="ps")
                        for kc in range(8):
                            nc.tensor.matmul(
                                ps,
                                wsb[:, kc],
                                xw[:, kc],
                                start=(kc == 0),
                                stop=(kc == 7),
                            )
                        state[key] = ps

                    return run

                def mk_rope(key, dest_fn):
                    def run():
                        ps = state.pop(key)
                        dest = dest_fn()
                        sh = rope_pool.tile([128, QSB], f32, tag="ropet", name="sh")
                        nc.vector.stream_shuffle(sh, ps, SWAP_MASK)
                        m1 = rope_pool.tile([128, QSB], f16, tag="ropet", name="m1")
                        nc.vector.tensor_mul(m1, ps, ct_sb[:, t0 : t0 + QSB])
                        m2 = rope_pool.tile([128, QSB], f16, tag="ropet", name="m2")
                        nc.vector.tensor_mul(m2, sh, st_sb[:, t0 : t0 + QSB])
                        nc.gpsimd.tensor_add(dest, m1, m2)

                    return run

                def mk_vpath():
                    def run():
                        ps = state.pop("v")
                        vt = vt_pool.tile([128, QSB], f16, name="vt")
                        nc.vector.tensor_copy(vt, ps)
                        vblk = proj_ps.tile([128, 4, 128], f16, tag="proj", name="vb")
                        for i in range(4):
                            nc.tensor.transpose(
                                vblk[:, i], vt[:, 128 * i : 128 * i + 128], ident_sb
                            )
                        for i in range(4):
                            tb = 4 * w + i
                            nc.vector.tensor_copy(VA[:, tb, 0:64], vblk[:, i, 0:64])
                            nc.vector.tensor_copy(
                                VA[:, tb, 65:129], vblk[:, i, 64:128]
                            )

                    return run

                quanta = [
                    mk_proj(wq_sb, "q"),
                    mk_rope("q", lambda: qt),
                    mk_proj(wk_sb, "k"),
                    mk_rope("k", lambda: KT[:, t0 : t0 + QSB]),
                    mk_proj(wv_sb, "v"),
                    mk_vpath(),
                ]
                return qt, quanta

            def emit_outproj(ctxn, b, t0):
                for ts in range(4):
                    if merge_out:
                        ops = out_ps.tile([128, 2, 512], f32, name="ops", tag="ops")
                        for eh in range(2):
                            nc.tensor.matmul(
                                ops[:, eh],
                                ctxn[:, 128 * ts : 128 * ts + 128],
                                wo_sb[:, 512 * eh : 512 * eh + 512],
                                start=True,
                                stop=True,
                            )
                        ost = ost_pool.tile([128, 2, 512], f16, name="ost", tag="ost")
                        if ts % 2 == 0:
                            nc.vector.tensor_copy(ost, ops)
                        else:
                            nc.scalar.copy(ost, ops)
                        nc.sync.dma_start(
                            out=out[b, t0 + 128 * ts : t0 + 128 * ts + 128, :],
                            in_=ost.rearrange("p a n -> p (a n)"),
                        )
                    else:
                        for eh in range(2):
                            ops = out_ps.tile([128, 512], f32, name="ops", tag="ops")
                            nc.tensor.matmul(
                                ops,
                                ctxn[:, 128 * ts : 128 * ts + 128],
                                wo_sb[:, 512 * eh : 512 * eh + 512],
                                start=True,
                                stop=True,
                            )
                            ost = ost_pool.tile([128, 512], f16, name="ost", tag="ost")
                            if eh == 0:
                                nc.vector.tensor_copy(ost, ops)
                            else:
                                nc.scalar.copy(ost, ops)
                            nc.sync.dma_start(
                                out=out[
                                    b,
                                    t0 + 128 * ts : t0 + 128 * ts + 128,
                                    512 * eh : 512 * eh + 512,
                                ],
                                in_=ost,
                            )

            seq = [(b, w) for b in range(B) for w in range(NW)]
            qts = {}
            qt0, quanta0 = prep_window(0, 0, first=(rep == 0))
            for q in quanta0:
                q()
            qts[(0, 0)] = qt0
            pending = None

            for i, (b, w) in enumerate(seq):
                t0 = w * QSB
                KT, VA = KTs[b], VAs[b]
                qt = qts.pop((b, w))

                nq = []
                if i + 1 < len(seq):
                    nb, nw = seq[i + 1]
                    qtn, nq = prep_window(nb, nw, first=False)
                    qts[(nb, nw)] = qtn
                if not pipeline:
                    for q in nq:
                        q()
                    nq = []

                nkb = 4 * w + 4
                cps = [
                    ctx_ps.tile([65, QSB], f32, tag="ctx", name=f"cps{h}")
                    for h in range(2)
                ]
                emitted = 0
                for kb in range(nkb):
                    o = kb - 4 * w
                    col0 = 128 * o if o > 0 else 0
                    ncols = QSB - col0
                    pts = []
                    for h in range(2):
                        sps = s_ps.tile([128, QSB], f32, tag="s", name="sps")
                        nc.tensor.matmul(
                            sps[:, :ncols],
                            KT[64 * h : 64 * h + 64, 128 * kb : 128 * kb + 128],
                            qt[64 * h : 64 * h + 64, col0:QSB],
                            start=True,
                            stop=True,
                        )
                        if o >= 0:
                            nc.vector.tensor_add(
                                sps[:, 0:128], sps[:, 0:128], mneg_sb
                            )
                        pt = p_pool.tile([128, QSB], f16, tag="pt", name="pt")
                        nc.scalar.activation(
                            pt[:, :ncols], sps[:, :ncols], AF.Exp, scale=0.125
                        )
                        pts.append(pt)
                    # interleave next-window projection quanta while ACT runs exp
                    want = (kb + 1) * len(nq) // nkb
                    while emitted < want:
                        nq[emitted]()
                        emitted += 1
                    for h in range(2):
                        nc.tensor.matmul(
                            cps[h][:, col0:QSB],
                            VA[:, kb, 65 * h : 65 * h + 65],
                            pts[h][:, :ncols],
                            start=(kb == 0),
                            stop=(kb == nkb - 1),
                        )
                while emitted < len(nq):
                    nq[emitted]()
                    emitted += 1

                # out-projection of the PREVIOUS window (deps long satisfied)
                if delay_out and pending is not None:
                    emit_outproj(*pending)
                    pending = None

                # normalize this window
                ctxn = ctxn_pool.tile([128, QSB], f16, name="ctxn")
                for h in range(2):
                    rc = nrm_pool.tile([65, QSB], f32, tag="rc", name="rc")
                    nc.vector.reciprocal(rc[64:65, :], cps[h][64:65, :])
                    rc0 = nrm_pool.tile([1, QSB], f32, tag="rc0", name="rc0")
                    nc.sync.dma_start(out=rc0, in_=rc[64:65, :])
                    bc = nrm_pool.tile([64, QSB], f32, tag="bc", name="bc")
                    nc.gpsimd.partition_broadcast(bc, rc0, channels=64)
                    if h == 0:
                        nc.vector.tensor_mul(ctxn[0:64, :], cps[h][0:64, :], bc)
                    else:
                        cn1 = nrm_pool.tile([64, QSB], f16, tag="cn1", name="cn1")
                        nc.vector.tensor_mul(cn1, cps[h][0:64, :], bc)
                        nc.sync.dma_start(out=ctxn[64:128, :], in_=cn1)
                if delay_out:
                    pending = (ctxn, b, t0)
                else:
                    emit_outproj(ctxn, b, t0)

            if pending is not None:
                emit_outproj(*pending)

    nc.compile()
    return nc


def _host_inputs(x, Wq, Wk, Wv, Wo):
    xT = np.ascontiguousarray(x.transpose(0, 2, 1)).astype(np.float16)

    pos = np.arange(T, dtype=np.float64)
    inv_freq = np.power(10000.0, -2.0 * np.arange(0, DH, 2) / DH)  # (32,)
    freqs = pos[:, None] * inv_freq[None, :]  # (T, 32)
    cos = np.cos(freqs)
    sin = np.sin(freqs)
    ct = np.empty((DC, T), np.float32)
    st = np.empty((DC, T), np.float32)
    for p in range(DC):
        i = (p % DH) // 2
        ct[p] = cos[:, i]
        st[p] = sin[:, i] * (-1.0 if p % 2 == 0 else 1.0)

    pp, cc = np.meshgrid(np.arange(128), np.arange(128), indexing="ij")
    mneg = np.where(pp <= cc, 0.0, -1e9).astype(np.float32)
    ident = np.eye(128, dtype=np.float32)

    per_core = []
    for c in range(NCORES):
        sl = slice(c * DC, (c + 1) * DC)
        per_core.append(
            {
                "xt": xT,
                "wq": np.ascontiguousarray(Wq[:, sl]).astype(np.float16),
                "wk": np.ascontiguousarray(Wk[:, sl]).astype(np.float16),
                "wv": np.ascontiguousarray(Wv[:, sl]).astype(np.float16),
                "wo": np.ascontiguousarray(Wo[sl, :]).astype(np.float16),
                "ropec": ct,
                "ropes": st,
                "mneg": mneg,
                "ident": ident.astype(np.float16),
                "vones": np.ones((128, 16), np.float16),
            }
        )
    return per_core


def kernel(x, Wq, Wk, Wv, Wo, bo):
    x = np.asarray(x, np.float32)
    Wq = np.asarray(Wq, np.float32)
    Wk = np.asarray(Wk, np.float32)
    Wv = np.asarray(Wv, np.float32)
    Wo = np.asarray(Wo, np.float32)
    bo = np.asarray(bo, np.float32)

    if "nc" not in _CACHE:
        _CACHE["nc"] = _build()
    nc = _CACHE["nc"]

    in_maps = _host_inputs(x, Wq, Wk, Wv, Wo)
    res = run_bass_kernel_spmd(nc, in_maps, list(range(NCORES)))
    acc = res.results[0]["out"].astype(np.float64)
    for c in range(1, NCORES):
        acc += res.results[c]["out"]
    acc += bo.astype(np.float64)
    return acc.astype(np.float32)



# revision 14
# speedup vs baseline: 3.1050x; 1.3314x over previous
"""Multi-head causal attention with RoPE on 8 Trainium2 NeuronCores.

Sharding: tensor-parallel over heads. Each core owns 2 of the 16 heads
(a 128-wide slice of D_OUT): it computes Q/K/V projections for its slice
(column-sliced Wq/Wk/Wv), RoPE, causal attention, and a row-sliced
out-projection partial. The 8 partials are summed on the host (the
all-reduce of the row-parallel out projection) and the bias added once.

Emission is software-pipelined for the in-order PE: the next window's
projection matmuls are interleaved between exp and ctx inside the
attention loop (covering ACT latency), and each window's out-projection
is delayed one window so PE never waits on the normalize chain.
"""

import sys

sys.path.insert(0, "/opt/trn_rl_repo")

from contextlib import ExitStack

import numpy as np

import concourse.bass as bass
import concourse.tile as tile
from concourse import bacc, mybir
from concourse.bass import MemorySpace
from concourse.bass_utils import run_bass_kernel_spmd

B, T, D, H, DH = 2, 2048, 1024, 16, 64
NCORES = 8
DC = D // NCORES  # 128: d-slice per core (2 heads)
QSB = 512  # query superblock
NW = T // QSB  # windows per batch

f32 = mybir.dt.float32
f32r = mybir.dt.float32r
f16 = mybir.dt.float16
AF = mybir.ActivationFunctionType

SWAP_MASK = []
for _i in range(16):
    SWAP_MASK += [2 * _i + 1, 2 * _i]

_CACHE = {}


def _build(reps=1, pipeline=True, delay_out=False, merge_out=False, act_frac=2, qt_bufs=2, rope_bufs=6, s_bufs=2, ctx_bufs=2, ost_bufs=4, p_bufs=4, proj_bufs=1):
    nc = bacc.Bacc("TRN2", target_bir_lowering=False, debug=False)
    xT = nc.dram_tensor("xt", [B, D, T], f16, kind="ExternalInput").ap()
    wq = nc.dram_tensor("wq", [D, DC], f16, kind="ExternalInput").ap()
    wk = nc.dram_tensor("wk", [D, DC], f16, kind="ExternalInput").ap()
    wv = nc.dram_tensor("wv", [D, DC], f16, kind="ExternalInput").ap()
    wo = nc.dram_tensor("wo", [DC, D], f16, kind="ExternalInput").ap()
    ropec = nc.dram_tensor("ropec", [DC, T], f32, kind="ExternalInput").ap()
    ropes = nc.dram_tensor("ropes", [DC, T], f32, kind="ExternalInput").ap()
    mneg = nc.dram_tensor("mneg", [128, 128], f32, kind="ExternalInput").ap()
    ident = nc.dram_tensor("ident", [128, 128], f16, kind="ExternalInput").ap()
    vones = nc.dram_tensor("vones", [128, 16], f16, kind="ExternalInput").ap()
    out = nc.dram_tensor("out", [B, T, D], f16, kind="ExternalOutput").ap()

    with tile.TileContext(nc) as tc, ExitStack() as ctx:
        const = ctx.enter_context(tc.tile_pool(name="const", bufs=1))
        x_pool = ctx.enter_context(tc.tile_pool(name="x_pool", bufs=2))
        kt_pool = ctx.enter_context(tc.tile_pool(name="kt_pool", bufs=2))
        va_pool = ctx.enter_context(tc.tile_pool(name="va_pool", bufs=2))
        qt_pool = ctx.enter_context(tc.tile_pool(name="qt_pool", bufs=qt_bufs))
        rope_pool = ctx.enter_context(tc.tile_pool(name="rope_pool", bufs=rope_bufs))
        vt_pool = ctx.enter_context(tc.tile_pool(name="vt_pool", bufs=2))
        p_pool = ctx.enter_context(tc.tile_pool(name="p_pool", bufs=p_bufs))
        nrm_pool = ctx.enter_context(tc.tile_pool(name="nrm_pool", bufs=2))
        ctxn_pool = ctx.enter_context(tc.tile_pool(name="ctxn_pool", bufs=2))
        ost_pool = ctx.enter_context(tc.tile_pool(name="ost_pool", bufs=ost_bufs))

        proj_ps = ctx.enter_context(
            tc.tile_pool(name="proj_ps", bufs=proj_bufs, space=MemorySpace.PSUM)
        )
        s_ps = ctx.enter_context(
            tc.tile_pool(name="s_ps", bufs=s_bufs, space=MemorySpace.PSUM)
        )
        ctx_ps = ctx.enter_context(
            tc.tile_pool(name="ctx_ps", bufs=ctx_bufs, space=MemorySpace.PSUM)
        )
        out_ps = ctx.enter_context(
            tc.tile_pool(name="out_ps", bufs=1, space=MemorySpace.PSUM)
        )

        # ---- constants (first-use order; first window's loads split) ----
        wq_sb = const.tile([128, 8, DC], f16)
        wk_sb = const.tile([128, 8, DC], f16)
        wv_sb = const.tile([128, 8, DC], f16)
        wqr = wq.rearrange("(c p) m -> p c m", p=128)
        xw0 = x_pool.tile([128, 8, QSB], f16, name="xw0", tag="xw")
        xr0 = xT[0, :, 0:QSB].rearrange("(c p) t -> p c t", p=128)
        for kc in range(8):
            nc.sync.dma_start(out=wq_sb[:, kc : kc + 1], in_=wqr[:, kc : kc + 1])
            nc.sync.dma_start(out=xw0[:, kc : kc + 1], in_=xr0[:, kc : kc + 1])
        nc.sync.dma_start(out=wk_sb, in_=wk.rearrange("(c p) m -> p c m", p=128))
        ct_sb = const.tile([128, T], f32)
        st_sb = const.tile([128, T], f32)
        nc.sync.dma_start(out=ct_sb, in_=ropec)
        nc.sync.dma_start(out=st_sb, in_=ropes)
        nc.sync.dma_start(out=wv_sb, in_=wv.rearrange("(c p) m -> p c m", p=128))
        ident_sb = const.tile([128, 128], f16)
        nc.sync.dma_start(out=ident_sb, in_=ident)
        mneg2_sb = const.tile([128, 2, 128], f32)
        nc.sync.dma_start(out=mneg2_sb[:, 0], in_=mneg)
        nc.sync.dma_start(out=mneg2_sb[:, 1], in_=mneg)
        wo_sb = const.tile([128, D], f16)
        nc.sync.dma_start(out=wo_sb, in_=wo)

        for rep in range(reps):
            KTs, VAs = {}, {}

            def prep_window(b, w, first):
                """Emit xw DMA now; return (qt_tile, quanta closures)."""
                t0 = w * QSB
                if w == 0:
                    KTs[b] = kt_pool.tile([128, T], f16, name=f"KT{b}", tag="KT")
                    VAs[b] = va_pool.tile(
                        [128, 16, 130], f16, name=f"VA{b}", tag="VA"
                    )
                    nc.sync.dma_start(
                        out=VAs[b][:, :, 64:65],
                        in_=vones.rearrange("p (c o) -> p c o", o=1),
                    )
                    nc.sync.dma_start(
                        out=VAs[b][:, :, 129:130],
                        in_=vones.rearrange("p (c o) -> p c o", o=1),
                    )
                KT, VA = KTs[b], VAs[b]
                if first:
                    xw = xw0
                else:
                    xw = x_pool.tile([128, 8, QSB], f16, name="xw", tag="xw")
                    nc.sync.dma_start(
                        out=xw,
                        in_=xT[b, :, t0 : t0 + QSB].rearrange(
                            "(c p) t -> p c t", p=128
                        ),
                    )
                qt = qt_pool.tile([128, QSB], f16, name="qt")
                state = {}

                def mk_proj(wsb, key):
                    def run():
                        ps = proj_ps.tile([128, QSB], f32, tag="proj", name="ps")
                        for kc in range(8):
                            nc.tensor.matmul(
                                ps,
                                wsb[:, kc],
                                xw[:, kc],
                                start=(kc == 0),
                                stop=(kc == 7),
                            )
                        state[key] = ps

                    return run

                def mk_rope(key, dest_fn):
                    def run():
                        ps = state.pop(key)
                        dest = dest_fn()
                        sh = rope_pool.tile([128, QSB], f32, tag="ropet", name="sh")
                        nc.vector.stream_shuffle(sh, ps, SWAP_MASK)
                        m1 = rope_pool.tile([128, QSB], f16, tag="ropet", name="m1")
                        nc.vector.tensor_mul(m1, ps, ct_sb[:, t0 : t0 + QSB])
                        m2 = rope_pool.tile([128, QSB], f16, tag="ropet", name="m2")
                        nc.vector.tensor_mul(m2, sh, st_sb[:, t0 : t0 + QSB])
                        nc.gpsimd.tensor_add(dest, m1, m2)

                    return run

                def mk_vpath():
                    def run():
                        ps = state.pop("v")
                        vt = vt_pool.tile([128, QSB], f16, name="vt")
                        nc.vector.tensor_copy(vt, ps)
                        vblk = proj_ps.tile([128, 4, 128], f16, tag="proj", name="vb")
                        for i in range(4):
                            nc.tensor.transpose(
                                vblk[:, i], vt[:, 128 * i : 128 * i + 128], ident_sb
                            )
                        for i in range(4):
                            tb = 4 * w + i
                            nc.vector.tensor_copy(VA[:, tb, 0:64], vblk[:, i, 0:64])
                            nc.vector.tensor_copy(
                                VA[:, tb, 65:129], vblk[:, i, 64:128]
                            )

                    return run

                quanta = [
                    mk_proj(wq_sb, "q"),
                    mk_rope("q", lambda: qt),
                    mk_proj(wk_sb, "k"),
                    mk_rope("k", lambda: KT[:, t0 : t0 + QSB]),
                    mk_proj(wv_sb, "v"),
                    mk_vpath(),
                ]
                return qt, quanta

            def emit_outproj(ctxn, b, t0):
                for ts in range(4):
                    if merge_out:
                        ops = out_ps.tile([128, 2, 512], f32, name="ops", tag="ops")
                        for eh in range(2):
                            nc.tensor.matmul(
                                ops[:, eh],
                                ctxn[:, 128 * ts : 128 * ts + 128],
                                wo_sb[:, 512 * eh : 512 * eh + 512],
                                start=True,
                                stop=True,
                            )
                        ost = ost_pool.tile([128, 2, 512], f16, name="ost", tag="ost")
                        if ts % 2 == 0:
                            nc.vector.tensor_copy(ost, ops)
                        else:
                            nc.scalar.copy(ost, ops)
                        nc.sync.dma_start(
                            out=out[b, t0 + 128 * ts : t0 + 128 * ts + 128, :],
                            in_=ost.rearrange("p a n -> p (a n)"),
                        )
                    else:
                        for eh in range(2):
                            ops = out_ps.tile([128, 512], f32, name="ops", tag="ops")
                            nc.tensor.matmul(
                                ops,
                                ctxn[:, 128 * ts : 128 * ts + 128],
                                wo_sb[:, 512 * eh : 512 * eh + 512],
                                start=True,
                                stop=True,
                            )
                            ost = ost_pool.tile([128, 512], f16, name="ost", tag="ost")
                            if eh == 0:
                                nc.vector.tensor_copy(ost, ops)
                            else:
                                nc.scalar.copy(ost, ops)
                            nc.sync.dma_start(
                                out=out[
                                    b,
                                    t0 + 128 * ts : t0 + 128 * ts + 128,
                                    512 * eh : 512 * eh + 512,
                                ],
                                in_=ost,
                            )

            seq = [(b, w) for b in range(B) for w in range(NW)]
            qts = {}
            qt0, quanta0 = prep_window(0, 0, first=(rep == 0))
            for q in quanta0:
                q()
            qts[(0, 0)] = qt0
            pending = None

            for i, (b, w) in enumerate(seq):
                t0 = w * QSB
                KT, VA = KTs[b], VAs[b]
                qt = qts.pop((b, w))

                nq = []
                if i + 1 < len(seq):
                    nb, nw = seq[i + 1]
                    qtn, nq = prep_window(nb, nw, first=False)
                    qts[(nb, nw)] = qtn
                if not pipeline:
                    for q in nq:
                        q()
                    nq = []

                nkb = 4 * w + 4
                cps = [
                    ctx_ps.tile([65, QSB], f32, tag="ctx", name=f"cps{h}")
                    for h in range(2)
                ]
                emitted = 0
                for kb in range(nkb):
                    o = kb - 4 * w
                    col0 = 128 * o if o > 0 else 0
                    ncols = QSB - col0
                    sps = s_ps.tile([128, 2, QSB], f32, tag="s", name="sps")
                    for h in range(2):
                        nc.tensor.matmul(
                            sps[:, h, :ncols],
                            KT[64 * h : 64 * h + 64, 128 * kb : 128 * kb + 128],
                            qt[64 * h : 64 * h + 64, col0:QSB],
                            start=True,
                            stop=True,
                        )
                    if o >= 0:
                        nc.vector.tensor_add(
                            sps[:, :, 0:128], sps[:, :, 0:128], mneg2_sb
                        )
                    pt = p_pool.tile([128, 2, QSB], f16, tag="pt", name="pt")
                    nc.scalar.activation(
                        pt[:, :, :ncols], sps[:, :, :ncols], AF.Exp, scale=0.125
                    )
                    # interleave next-window projection quanta while ACT runs exp
                    want = (kb + 1) * len(nq) // nkb
                    while emitted < want:
                        nq[emitted]()
                        emitted += 1
                    for h in range(2):
                        nc.tensor.matmul(
                            cps[h][:, col0:QSB],
                            VA[:, kb, 65 * h : 65 * h + 65],
                            pt[:, h, :ncols],
                            start=(kb == 0),
                            stop=(kb == nkb - 1),
                        )
                while emitted < len(nq):
                    nq[emitted]()
                    emitted += 1

                # out-projection of the PREVIOUS window (deps long satisfied)
                if delay_out and pending is not None:
                    emit_outproj(*pending)
                    pending = None

                # normalize this window
                ctxn = ctxn_pool.tile([128, QSB], f16, name="ctxn")
                for h in range(2):
                    rc = nrm_pool.tile([65, QSB], f32, tag="rc", name="rc")
                    nc.vector.reciprocal(rc[64:65, :], cps[h][64:65, :])
                    rc0 = nrm_pool.tile([1, QSB], f32, tag="rc0", name="rc0")
                    nc.sync.dma_start(out=rc0, in_=rc[64:65, :])
                    bc = nrm_pool.tile([64, QSB], f32, tag="bc", name="bc")
                    nc.gpsimd.partition_broadcast(bc, rc0, channels=64)
                    if h == 0:
                        nc.vector.tensor_mul(ctxn[0:64, :], cps[h][0:64, :], bc)
                    else:
                        cn1 = nrm_pool.tile([64, QSB], f16, tag="cn1", name="cn1")
                        nc.vector.tensor_mul(cn1, cps[h][0:64, :], bc)
                        nc.sync.dma_start(out=ctxn[64:128, :], in_=cn1)
                if delay_out:
                    pending = (ctxn, b, t0)
                else:
                    emit_outproj(ctxn, b, t0)

            if pending is not None:
                emit_outproj(*pending)

    nc.compile()
    return nc


def _host_inputs(x, Wq, Wk, Wv, Wo):
    xT = np.ascontiguousarray(x.transpose(0, 2, 1)).astype(np.float16)

    pos = np.arange(T, dtype=np.float64)
    inv_freq = np.power(10000.0, -2.0 * np.arange(0, DH, 2) / DH)  # (32,)
    freqs = pos[:, None] * inv_freq[None, :]  # (T, 32)
    cos = np.cos(freqs)
    sin = np.sin(freqs)
    ct = np.empty((DC, T), np.float32)
    st = np.empty((DC, T), np.float32)
    for p in range(DC):
        i = (p % DH) // 2
        ct[p] = cos[:, i]
        st[p] = sin[:, i] * (-1.0 if p % 2 == 0 else 1.0)

    pp, cc = np.meshgrid(np.arange(128), np.arange(128), indexing="ij")
    mneg = np.where(pp <= cc, 0.0, -1e9).astype(np.float32)
    ident = np.eye(128, dtype=np.float32)

    per_core = []
    for c in range(NCORES):
        sl = slice(c * DC, (c + 1) * DC)
        per_core.append(
            {
                "xt": xT,
                "wq": np.ascontiguousarray(Wq[:, sl]).astype(np.float16),
                "wk": np.ascontiguousarray(Wk[:, sl]).astype(np.float16),
                "wv": np.ascontiguousarray(Wv[:, sl]).astype(np.float16),
                "wo": np.ascontiguousarray(Wo[sl, :]).astype(np.float16),
                "ropec": ct,
                "ropes": st,
                "mneg": mneg,
                "ident": ident.astype(np.float16),
                "vones": np.ones((128, 16), np.float16),
            }
        )
    return per_core


def kernel(x, Wq, Wk, Wv, Wo, bo):
    x = np.asarray(x, np.float32)
    Wq = np.asarray(Wq, np.float32)
    Wk = np.asarray(Wk, np.float32)
    Wv = np.asarray(Wv, np.float32)
    Wo = np.asarray(Wo, np.float32)
    bo = np.asarray(bo, np.float32)

    if "nc" not in _CACHE:
        _CACHE["nc"] = _build()
    nc = _CACHE["nc"]

    in_maps = _host_inputs(x, Wq, Wk, Wv, Wo)
    res = run_bass_kernel_spmd(nc, in_maps, list(range(NCORES)))
    acc = res.results[0]["out"].astype(np.float64)
    for c in range(1, NCORES):
        acc += res.results[c]["out"]
    acc += bo.astype(np.float64)
    return acc.astype(np.float32)

